# revision 3
# baseline (speedup 1.0000x reference)
"""Trainium2 Bass kernel for the MAB (multihead attention block) problem.

Full inputs in, full outputs out. Data-parallel over batch: 16 batches
across 8 NeuronCores = 2 batches/core. No collectives.

Per-core pipeline (per batch):
  1. QpT/KpT = (Q @ Wq)^T etc in bf16; Vp natural augmented with a ones
     column per head (softmax denominator rides the PV matmul).
  2. Attention in S^T layout: per (q-half hf, head-pair hp, k-tile m):
     S^T[k,q] = Kh @ Qh^T, P = exp(S^T*s) on the scalar engine
     ([128,1024] tiles, no max subtraction -- scores are N(0,~0.35)).
     PV in NATURAL layout: O[q,0:65] += P_chunk[k,q]^T @ [Vh|1][k,0:65]
     accumulated over the 8 k-tiles (col 64 = softmax denominator).
     Drain: batched reciprocal + one STT per (head, qtile) fusing the
     1/den scaling with the Qp residual (oasm is prefilled with Qp via
     PE transposes of QpT, so no separate qp buffer).
  3. LayerNorm (stats on DVE, apply on GpSimd) -> bf16 transpose ->
     FFN matmul -> relu+residual fused in one DVE STT -> LayerNorm ->
     DMA out.
  4. The program is emitted as one software-pipelined stream: attention
     is exp(ACT)-bound, so projection/transpose/FFN "filler" chunks are
     interleaved into the PE stream via a token-bucket pump. A keyed
     chunk registry with need() forcing keeps emission order consistent
     with dataflow (the tile framework resolves deps by emission order).

Affine/bias params that are identically (1, 0) are folded out at build
time (checked against the actual input values).
"""

import math
import sys
from collections import deque
from contextlib import ExitStack

import numpy as np

sys.path.insert(0, "/opt/trn_rl_repo")

import concourse.bass as bass
import concourse.tile as tile
from concourse import bacc
from concourse import mybir
from concourse.bass import ds, ts
from concourse.bass_utils import run_bass_kernel_spmd
from concourse.masks import make_identity

FP = mybir.dt.float32
BF = mybir.dt.bfloat16
AF = mybir.ActivationFunctionType
ALU = mybir.AluOpType

B, N, D = 16, 1024, 512
NCORES = 8
BL = B // NCORES
H, HD, HA = 8, 64, 65
PAIRS = H // 2
SCALE = 1.0 / math.sqrt(D)
EPS = 1e-5
P = 128
DT = D // P  # 4 dv chunks
NT = N // P  # 8 n tiles
QH = NT // 2  # 4 qtiles per q-half

# pipeline pump pacing (ns): estimated PE idle per attention m-step and
# PE cost of one filler chunk. Only affects emission interleaving.
STEP_IDLE_NS = 420.0
CHUNK_NS = 880.0


def _bcast_ap(ap):
    """Broadcast a [D]-shaped DRAM AP across all 128 partitions."""
    return bass.AP(tensor=ap.tensor, offset=ap.offset, ap=[[0, P]] + list(ap.ap))


def _build_program(tbq, tbk, tbv, trivbo, triv0, triv1):
    nc = bacc.Bacc(None, target_bir_lowering=False)
    dr = {}
    for name, shape in [
        ("QT", [BL, D, N]),
        ("KT", [BL, D, N]),
        ("Wq", [D, D]),
        ("Wk", [D, D]),
        ("Wv", [D, D]),
        ("Wo", [D, D]),
        ("bq2", [P, DT]),
        ("bk2", [P, DT]),
        ("bv", [D]),
        ("bo", [D]),
        ("g0", [D]),
        ("b0", [D]),
        ("g1", [D]),
        ("b1", [D]),
    ]:
        dt = BF if name in ("QT", "KT", "Wq", "Wk", "Wv", "Wo") else FP
        dr[name] = nc.declare_dram_parameter(name, shape, dt, isOutput=False)
    out_O = nc.declare_dram_parameter("O", [BL, N, D], FP, isOutput=True)

    qt_src = dr["QT"][:].rearrange("b (c p) n -> b p c n", p=P)
    kt_src = dr["KT"][:].rearrange("b (c p) n -> b p c n", p=P)

    with tile.TileContext(nc) as tc, ExitStack() as ctx:
        singles = ctx.enter_context(tc.tile_pool(name="singles", bufs=1))
        io = ctx.enter_context(tc.tile_pool(name="io", bufs=1))
        big = ctx.enter_context(tc.tile_pool(name="big", bufs=2))
        pch = ctx.enter_context(tc.tile_pool(name="pch", bufs=6))
        sml = ctx.enter_context(tc.tile_pool(name="sml", bufs=10))
        two = ctx.enter_context(tc.tile_pool(name="two", bufs=2))
        # PSUM budget: flow 2 bufs x 2 banks + pv 3 x 1 + acc 1 x 1 = 8
        ps_flow = ctx.enter_context(tc.tile_pool(name="ps_flow", bufs=2, space="PSUM"))
        ps_pv = ctx.enter_context(tc.tile_pool(name="ps_pv", bufs=3, space="PSUM"))
        ps_acc = ctx.enter_context(tc.tile_pool(name="ps_acc", bufs=1, space="PSUM"))

        # ---------------- statics ----------------
        wsb = {}
        for wname in ("Wq", "Wk", "Wv", "Wo"):
            wsb[wname] = singles.tile([P, DT, D], BF, tag=wname, name=wname)
        for wname in ("Wk", "Wq", "Wv"):
            nc.sync.dma_start(
                out=wsb[wname], in_=dr[wname][:].rearrange("(c p) d -> p c d", p=P)
            )
        ident_f = singles.tile([P, P], FP, tag="identf")
        make_identity(nc, ident_f)
        ident_bf = singles.tile([P, P], BF, tag="identbf")
        nc.vector.tensor_copy(ident_bf, ident_f)
        eps_sb = singles.tile([P, 1], FP, tag="eps")
        nc.vector.memset(eps_sb, EPS)

        bq_sb = bk_sb = None
        if not tbq:
            bq_sb = singles.tile([P, DT], FP, tag="bq2")
            nc.sync.dma_start(out=bq_sb, in_=dr["bq2"][:])
        if not tbk:
            bk_sb = singles.tile([P, DT], FP, tag="bk2")
            nc.sync.dma_start(out=bk_sb, in_=dr["bk2"][:])
        bc = {}
        for bname, trivial in (
            ("bv", tbv),
            ("bo", trivbo),
            ("g0", triv0),
            ("b0", triv0),
            ("g1", triv1),
            ("b1", triv1),
        ):
            if not trivial:
                t = singles.tile([P, D], FP, tag=bname)
                nc.gpsimd.dma_start(out=t, in_=_bcast_ap(dr[bname][:]))
                bc[bname] = t

        # ---------------- chunk registry + pump ----------------
        chunks = {}
        fillq = deque()
        deficit = [0.0]

        def reg(key, fn):
            chunks[key] = fn
            fillq.append(key)

        def need(key):
            fn = chunks.pop(key, None)
            if fn is not None:
                fn()

        def pump(ns):
            deficit[0] += ns
            while deficit[0] >= CHUNK_NS and fillq:
                key = fillq.popleft()
                fn = chunks.pop(key, None)
                if fn is None:
                    continue  # already forced via need()
                fn()
                deficit[0] -= CHUNK_NS

        def drainq():
            while fillq:
                key = fillq.popleft()
                fn = chunks.pop(key, None)
                if fn is not None:
                    fn()

        tiles = {}

        # ---------------- phase A: projections ----------------
        def dma_in(b):
            qt = io.tile([P, DT, N], BF, tag="qt", name="qt")
            kt = io.tile([P, DT, N], BF, tag="kt", name="kt")
            for c in range(DT):
                nc.sync.dma_start(out=kt[:, c, :], in_=kt_src[b, :, c, :])
            for c in range(DT):
                nc.sync.dma_start(out=qt[:, c, :], in_=qt_src[b, :, c, :])
            st = tiles.setdefault(b, {})
            st["qt"], st["kt"] = qt, kt

        def make_batch_tiles(b):
            st = tiles[b]
            st["qpt"] = big.tile([P, DT, N], BF, tag="qpt", name="qpt")
            st["kpt"] = big.tile([P, DT, N], BF, tag="kpt", name="kpt")
            st["vpa"] = big.tile([P, NT, H, HA], BF, tag="vpa", name="vpa")
            st["oasm"] = big.tile([P, NT, D], FP, tag="oasm", name="oasm")
            st["ln1"] = big.tile([P, NT, D], BF, tag="ln1", name="ln1")
            # denominator ones columns (evictions never touch col 64)
            nc.gpsimd.memset(st["vpa"][:, :, :, HD:HA], 1.0)

        def proj_chunk(b, wname, dst_name, bias_sb, t, hf):
            """One [128,512] tile of QpT/KpT: 4 matmuls + eviction."""

            def run():
                st = tiles[b]
                src = st["qt"] if wname == "Wq" else st["kt"]
                ps = ps_acc.tile([P, D], FP, tag="acc", name="projps")
                for c in range(DT):
                    nc.tensor.matmul(
                        ps,
                        wsb[wname][:, c, ts(t, P)],
                        src[:, c, ds(hf * 512, 512)],
                        start=(c == 0),
                        stop=(c == DT - 1),
                    )
                dst = st[dst_name][:, t, ds(hf * 512, 512)]
                if bias_sb is None:
                    nc.vector.tensor_copy(dst, ps)
                else:
                    nc.vector.tensor_scalar_add(dst, ps, bias_sb[:, t : t + 1])

            return run

        def vpa_chunk(b, m):
            def run():
                st = tiles[b]
                ps = ps_acc.tile([P, D], FP, tag="acc", name="vps")
                for c in range(DT):
                    nc.tensor.matmul(
                        ps,
                        st["kt"][:, c, ts(m, P)],
                        wsb["Wv"][:, c, :],
                        start=(c == 0),
                        stop=(c == DT - 1),
                    )
                vdst = st["vpa"][:, m, :, 0:HD]
                vsrc = ps[:, :].rearrange("p (h s) -> p h s", s=HD)
                if tbv:
                    nc.vector.tensor_copy(vdst, vsrc)
                else:
                    nc.vector.scalar_tensor_tensor(
                        out=vdst,
                        in0=vsrc,
                        scalar=0.0,
                        in1=bc["bv"][:, :].rearrange("p (h s) -> p h s", s=HD),
                        op0=ALU.bypass,
                        op1=ALU.add,
                    )

            return run

        def qp_chunk(b, m):
            """Prefill oasm[:, m, :] with natural-layout Qp (residual)."""

            def run():
                st = tiles[b]
                tp = ps_acc.tile([P, D], BF, tag="acc", name="qptr")
                for t in range(DT):
                    nc.tensor.transpose(
                        tp[:, ts(t, P)], st["qpt"][:, t, ts(m, P)], ident_bf
                    )
                nc.vector.tensor_copy(st["oasm"][:, m, :], tp)

            return run

        def phase_a_chunks(b):
            """(crit, rest) keyed chunk lists in need-order for batch b."""
            pq = lambda t, hf: (("qpt", b, t, hf), proj_chunk(b, "Wq", "qpt", bq_sb, t, hf))
            pk = lambda t, hf: (("kpt", b, t, hf), proj_chunk(b, "Wk", "kpt", bk_sb, t, hf))
            va = lambda m: (("va", b, m), vpa_chunk(b, m))
            qp = lambda m: (("qp", b, m), qp_chunk(b, m))
            crit = [pk(0, 0), pk(0, 1), pq(0, 0), va(0), va(1), va(2)]
            rest = [
                va(3),
                pq(1, 0),
                va(4),
                pq(2, 0),
                va(5),
                pq(3, 0),
                va(6),
                pk(1, 0),
                pk(1, 1),
                va(7),
                pk(2, 0),
                pk(2, 1),
                pk(3, 0),
                pk(3, 1),
                pq(0, 1),
                pq(1, 1),
                pq(2, 1),
                pq(3, 1),
                qp(0),
                qp(1),
                qp(2),
                qp(3),
                qp(4),
                qp(5),
                qp(6),
                qp(7),
            ]
            return crit, rest

        def force_qtkt_readers(b):
            """Emit all still-pending chunks that read qt/kt of batch b."""
            for key in [("kpt", b, t, hf) for t in range(DT) for hf in range(2)] + [
                ("qpt", b, t, hf) for t in range(DT) for hf in range(2)
            ] + [("va", b, m) for m in range(NT)]:
                need(key)

        # ---------------- LN1 + FFN ----------------
        def ln1_half(b, hf):
            st = tiles[b]
            mv = sml.tile([P, QH, 2], FP, tag="mva", name="mva")
            for i in range(QH):
                q = hf * QH + i
                bn = sml.tile([P, 6], FP, tag="bn", name="bn1")
                nc.vector.bn_stats(bn, st["oasm"][:, q, :])
                nc.vector.bn_aggr(mv[:, i, :], bn)
            rsa = sml.tile([P, QH], FP, tag="rsa", name="rsa")
            nc.scalar.activation(rsa, mv[:, :, 1], AF.Sqrt, bias=eps_sb)
            nc.vector.reciprocal(rsa, rsa)
            for i in range(QH):
                q = hf * QH + i
                lq = st["ln1"][:, q, :]
                nc.gpsimd.tensor_scalar(
                    out=lq,
                    in0=st["oasm"][:, q, :],
                    scalar1=mv[:, i, 0:1],
                    scalar2=rsa[:, i : i + 1],
                    op0=ALU.subtract,
                    op1=ALU.mult,
                )
                if not triv0:
                    nc.gpsimd.tensor_tensor(lq, lq, bc["g0"], ALU.mult)
                    nc.gpsimd.tensor_tensor(lq, lq, bc["b0"], ALU.add)

        def ffn_chunk(b, q):
            def run():
                st = tiles[b]
                tp = ps_acc.tile([P, D], BF, tag="acc", name="lntr")
                for c in range(DT):
                    nc.tensor.transpose(
                        tp[:, ts(c, P)], st["ln1"][:, q, ts(c, P)], ident_bf
                    )
                l_t = two.tile([P, DT, P], BF, tag="lt", name="lt")
                nc.vector.tensor_copy(l_t, tp)

                f_ps = ps_acc.tile([P, D], FP, tag="acc", name="ffps")
                for c in range(DT):
                    nc.tensor.matmul(
                        f_ps,
                        l_t[:, c, :],
                        wsb["Wo"][:, c, :],
                        start=(c == 0),
                        stop=(c == DT - 1),
                    )
                o2 = two.tile([P, D], FP, tag="o2", name="o2")
                if trivbo:
                    # o2 = relu(ffn) + ln1 in one STT
                    nc.vector.scalar_tensor_tensor(
                        out=o2,
                        in0=f_ps,
                        scalar=0.0,
                        in1=st["ln1"][:, q, :],
                        op0=ALU.max,
                        op1=ALU.add,
                    )
                else:
                    rl = two.tile([P, D], FP, tag="rl", name="rl")
                    nc.vector.tensor_tensor(rl, f_ps, bc["bo"], ALU.add)
                    nc.scalar.activation(rl, rl, AF.Relu)
                    nc.vector.tensor_tensor(o2, rl, st["ln1"][:, q, :], ALU.add)

                bn = sml.tile([P, 6], FP, tag="bn", name="bn2")
                nc.vector.bn_stats(bn, o2)
                mv2 = sml.tile([P, 2], FP, tag="mv2", name="mv2")
                nc.vector.bn_aggr(mv2, bn)
                rs2 = sml.tile([P, 1], FP, tag="rs2", name="rs2")
                nc.scalar.activation(rs2, mv2[:, 1:2], AF.Sqrt, bias=eps_sb)
                nc.vector.reciprocal(rs2, rs2)
                z2 = two.tile([P, D], FP, tag="z2", name="z2")
                nc.gpsimd.tensor_scalar(
                    out=z2,
                    in0=o2,
                    scalar1=mv2[:, 0:1],
                    scalar2=rs2,
                    op0=ALU.subtract,
                    op1=ALU.mult,
                )
                if not triv1:
                    nc.gpsimd.tensor_tensor(z2, z2, bc["g1"], ALU.mult)
                    nc.gpsimd.tensor_tensor(z2, z2, bc["b1"], ALU.add)
                nc.sync.dma_start(out=out_O[b, ts(q, P), :], in_=z2)

            return run

        # ---------------- phase B: attention ----------------
        def phase_b(b, at_hf0_end=None):
            st = tiles[b]
            qpt, kpt, vpa, oasm = st["qpt"], st["kpt"], st["vpa"], st["oasm"]
            pending = deque()  # (emit, p_tile, m)
            pending_drain = [None]

            def pop_pv():
                emit, p, m = pending.popleft()
                need(("va", b, m))
                emit(p, m)

            def mk_drain(hf, hp, pv0, pv1):
                def d():
                    for i in range(QH):
                        need(("qp", b, hf * QH + i))
                    for j, pv in ((0, pv0), (1, pv1)):
                        h = 2 * hp + j
                        r4 = sml.tile([P, QH], FP, tag="r4", name="r4")
                        nc.vector.reciprocal(r4, pv[:, :, HD:HA])
                        for qt in range(QH):
                            o_slice = oasm[:, hf * QH + qt, ds(h * HD, HD)]
                            nc.vector.scalar_tensor_tensor(
                                out=o_slice,
                                in0=pv[:, qt, 0:HD],
                                scalar=r4[:, qt : qt + 1],
                                in1=o_slice,
                                op0=ALU.mult,
                                op1=ALU.add,
                            )

                return d

            for hf in range(2):
                for hp in range(PAIRS):
                    need(("kpt", b, hp, 0))
                    need(("kpt", b, hp, 1))
                    need(("qpt", b, hp, hf))
                    pv0 = ps_pv.tile([P, QH, HA], FP, tag="pv", name="pv0")
                    pv1 = ps_pv.tile([P, QH, HA], FP, tag="pv", name="pv1")

                    def mk_emit(pv0=pv0, pv1=pv1, hp=hp):
                        def emit(p, m):
                            for j, pv in ((0, pv0), (1, pv1)):
                                for qt in range(QH):
                                    nc.tensor.matmul(
                                        pv[:, qt, :],
                                        p[:, ds(j * 512 + qt * P, P)],
                                        vpa[:, m, 2 * hp + j, :],
                                        start=(m == 0),
                                        stop=(m == NT - 1),
                                    )

                        return emit

                    emit = mk_emit()
                    for m in range(NT):
                        s = ps_flow.tile([P, N], FP, tag="flow", name="spair")
                        for j in range(2):
                            lo = j * HD
                            nc.tensor.matmul(
                                s[:, ds(j * 512, 512)],
                                kpt[lo : lo + HD, hp, ts(m, P)],
                                qpt[lo : lo + HD, hp, ds(hf * 512, 512)],
                                start=True,
                                stop=True,
                            )
                        p = pch.tile([P, N], BF, tag="p", name="p")
                        nc.scalar.activation(p, s, AF.Exp, scale=SCALE)
                        pending.append((emit, p, m))
                        if len(pending) > 2:
                            pop_pv()
                        if m == 1 and pending_drain[0] is not None:
                            pending_drain[0]()
                            pending_drain[0] = None
                        pump(STEP_IDLE_NS)
                    pending_drain[0] = mk_drain(hf, hp, pv0, pv1)

                # ---- end of this q-half: flush pipeline, drain, LN1, FFN
                while pending:
                    pop_pv()
                pending_drain[0]()
                pending_drain[0] = None
                ln1_half(b, hf)
                for i in range(QH):
                    q = hf * QH + i
                    reg(("ffn", b, q), ffn_chunk(b, q))
                if hf == 0 and at_hf0_end is not None:
                    at_hf0_end()

        # ---------------- main flow ----------------
        dma_in(0)
        nc.sync.dma_start(
            out=wsb["Wo"], in_=dr["Wo"][:].rearrange("(c p) d -> p c d", p=P)
        )
        make_batch_tiles(0)
        crit0, rest0 = phase_a_chunks(0)
        for _key, fn in crit0:
            fn()
        for key, fn in rest0:
            reg(key, fn)

        def start_b1():
            force_qtkt_readers(0)
            dma_in(1)
            make_batch_tiles(1)
            crit1, rest1 = phase_a_chunks(1)
            for key, fn in crit1 + rest1:
                reg(key, fn)

        phase_b(0, at_hf0_end=start_b1)
        phase_b(1)
        drainq()

    nc.compile()
    return nc


_NC = {}


def _get_nc(key):
    if key not in _NC:
        _NC[key] = _build_program(*key)
    return _NC[key]


def _prep_in_maps(inputs):
    import ml_dtypes

    f32 = lambda x: np.ascontiguousarray(np.asarray(x), dtype=np.float32)

    def bf(x):
        return np.ascontiguousarray(
            np.asarray(x, dtype=np.float32).astype(ml_dtypes.bfloat16)
        )

    Q, K = f32(inputs["Q"]), f32(inputs["K"])
    QT = np.ascontiguousarray(Q.transpose(0, 2, 1))
    KT = np.ascontiguousarray(K.transpose(0, 2, 1))
    shared = {
        "Wq": bf(inputs["Wq"]),
        "Wk": bf(inputs["Wk"]),
        "Wv": bf(inputs["Wv"]),
        "Wo": bf(inputs["Wo"]),
        "bq2": np.ascontiguousarray(f32(inputs["bq"]).reshape(DT, P).T),
        "bk2": np.ascontiguousarray(f32(inputs["bk"]).reshape(DT, P).T),
        "bv": f32(inputs["bv"]),
        "bo": f32(inputs["bo"]),
        "g0": f32(inputs["g0"]),
        "b0": f32(inputs["b0"]),
        "g1": f32(inputs["g1"]),
        "b1": f32(inputs["b1"]),
    }
    in_maps = []
    for c in range(NCORES):
        m = dict(shared)
        m["QT"] = np.ascontiguousarray(
            QT[c * BL : (c + 1) * BL].astype(ml_dtypes.bfloat16)
        )
        m["KT"] = np.ascontiguousarray(
            KT[c * BL : (c + 1) * BL].astype(ml_dtypes.bfloat16)
        )
        in_maps.append(m)
    return in_maps


def _run(inputs, trace=False):
    a = np.asarray
    key = (
        bool(np.all(a(inputs["bq"]) == 0.0)),
        bool(np.all(a(inputs["bk"]) == 0.0)),
        bool(np.all(a(inputs["bv"]) == 0.0)),
        bool(np.all(a(inputs["bo"]) == 0.0)),
        bool(np.all(a(inputs["g0"]) == 1.0) and np.all(a(inputs["b0"]) == 0.0)),
        bool(np.all(a(inputs["g1"]) == 1.0) and np.all(a(inputs["b1"]) == 0.0)),
    )
    nc = _get_nc(key)
    in_maps = _prep_in_maps(inputs)
    return run_bass_kernel_spmd(nc, in_maps, list(range(NCORES)), trace=trace)


def kernel(**inputs):
    res = _run(inputs, trace=False)
    return np.concatenate([res.results[c]["O"] for c in range(NCORES)], axis=0)


# revision 5
# speedup vs baseline: 1.5279x; 1.5279x over previous
"""Trainium2 Bass kernel for the MAB (multihead attention block) problem.

Full inputs in, full outputs out. Data-parallel over batch: 16 batches
across 8 NeuronCores = 2 batches/core. No collectives.

Per-core pipeline (per batch):
  1. QpT/KpT = (Q @ Wq)^T etc in bf16; Vp natural augmented with a ones
     column per head (softmax denominator rides the PV matmul).
  2. Attention in S^T layout: per (q-half hf, head-pair hp, k-tile m):
     S^T[k,q] = Kh @ Qh^T, P = exp(S^T*s) on the scalar engine
     ([128,1024] tiles, no max subtraction -- scores are N(0,~0.35)).
     PV in NATURAL layout: O[q,0:65] += P_chunk[k,q]^T @ [Vh|1][k,0:65]
     accumulated over the 8 k-tiles (col 64 = softmax denominator).
     Drain: batched reciprocal + one STT per (head, qtile) fusing the
     1/den scaling with the Qp residual (oasm is prefilled with Qp via
     PE transposes of QpT, so no separate qp buffer).
  3. LayerNorm (stats on DVE, apply on GpSimd) -> bf16 transpose ->
     FFN matmul -> relu+residual fused in one DVE STT -> LayerNorm ->
     DMA out.
  4. The program is emitted as one software-pipelined stream: attention
     is exp(ACT)-bound, so projection/transpose/FFN "filler" chunks are
     interleaved into the PE stream via a token-bucket pump. A keyed
     chunk registry with need() forcing keeps emission order consistent
     with dataflow (the tile framework resolves deps by emission order).

Affine/bias params that are identically (1, 0) are folded out at build
time (checked against the actual input values).
"""

import math
import sys
from collections import deque
from contextlib import ExitStack

import numpy as np

sys.path.insert(0, "/opt/trn_rl_repo")

import concourse.bass as bass
import concourse.tile as tile
from concourse import bacc
from concourse import mybir
from concourse.bass import ds, ts
from concourse.bass_utils import run_bass_kernel_spmd
from concourse.masks import make_identity

FP = mybir.dt.float32
BF = mybir.dt.bfloat16
AF = mybir.ActivationFunctionType
ALU = mybir.AluOpType

B, N, D = 16, 1024, 512
NCORES = 8
BL = B // NCORES
H, HD, HA = 8, 64, 65
PAIRS = H // 2
SCALE = 1.0 / math.sqrt(D)
EPS = 1e-5
P = 128
DT = D // P  # 4 dv chunks
NT = N // P  # 8 n tiles
QH = NT // 2  # 4 qtiles per q-half

# pipeline pump pacing (ns): estimated PE idle per attention m-step and
# PE cost of one filler chunk. Only affects emission interleaving.
STEP_IDLE_NS = 420.0
CHUNK_NS = 880.0


def _bcast_ap(ap):
    """Broadcast a [D]-shaped DRAM AP across all 128 partitions."""
    return bass.AP(tensor=ap.tensor, offset=ap.offset, ap=[[0, P]] + list(ap.ap))


def _build_program(tbq, tbk, tbv, trivbo, triv0, triv1):
    nc = bacc.Bacc(None, target_bir_lowering=False)
    dr = {}
    for name, shape in [
        ("QT", [BL, D, N]),
        ("KT", [BL, D, N]),
        ("Wq", [D, D]),
        ("Wk", [D, D]),
        ("Wv", [D, D]),
        ("Wo", [D, D]),
        ("bq2", [P, DT]),
        ("bk2", [P, DT]),
        ("bv", [D]),
        ("bo", [D]),
        ("g0", [D]),
        ("b0", [D]),
        ("g1", [D]),
        ("b1", [D]),
    ]:
        dt = BF if name in ("QT", "KT", "Wq", "Wk", "Wv", "Wo") else FP
        dr[name] = nc.declare_dram_parameter(name, shape, dt, isOutput=False)
    out_O = nc.declare_dram_parameter("O", [BL, N, D], FP, isOutput=True)

    qt_src = dr["QT"][:].rearrange("b (c p) n -> b p c n", p=P)
    kt_src = dr["KT"][:].rearrange("b (c p) n -> b p c n", p=P)

    with tile.TileContext(nc) as tc, ExitStack() as ctx:
        singles = ctx.enter_context(tc.tile_pool(name="singles", bufs=1))
        io = ctx.enter_context(tc.tile_pool(name="io", bufs=1))
        big = ctx.enter_context(tc.tile_pool(name="big", bufs=2))
        pch = ctx.enter_context(tc.tile_pool(name="pch", bufs=6))
        sml = ctx.enter_context(tc.tile_pool(name="sml", bufs=10))
        two = ctx.enter_context(tc.tile_pool(name="two", bufs=2))
        # PSUM budget: flow 2 bufs x 2 banks + pv 3 x 1 + acc 1 x 1 = 8
        ps_flow = ctx.enter_context(tc.tile_pool(name="ps_flow", bufs=2, space="PSUM"))
        ps_pv = ctx.enter_context(tc.tile_pool(name="ps_pv", bufs=3, space="PSUM"))
        ps_acc = ctx.enter_context(tc.tile_pool(name="ps_acc", bufs=1, space="PSUM"))

        # ---------------- statics ----------------
        wsb = {}
        for wname in ("Wq", "Wk", "Wv", "Wo"):
            wsb[wname] = singles.tile([P, DT, D], BF, tag=wname, name=wname)
        for wname in ("Wk", "Wq", "Wv"):
            nc.sync.dma_start(
                out=wsb[wname], in_=dr[wname][:].rearrange("(c p) d -> p c d", p=P)
            )
        ident_f = singles.tile([P, P], FP, tag="identf")
        make_identity(nc, ident_f)
        ident_bf = singles.tile([P, P], BF, tag="identbf")
        nc.vector.tensor_copy(ident_bf, ident_f)
        eps_sb = singles.tile([P, 1], FP, tag="eps")
        nc.vector.memset(eps_sb, EPS)

        bq_sb = bk_sb = None
        if not tbq:
            bq_sb = singles.tile([P, DT], FP, tag="bq2")
            nc.sync.dma_start(out=bq_sb, in_=dr["bq2"][:])
        if not tbk:
            bk_sb = singles.tile([P, DT], FP, tag="bk2")
            nc.sync.dma_start(out=bk_sb, in_=dr["bk2"][:])
        bc = {}
        for bname, trivial in (
            ("bv", tbv),
            ("bo", trivbo),
            ("g0", triv0),
            ("b0", triv0),
            ("g1", triv1),
            ("b1", triv1),
        ):
            if not trivial:
                t = singles.tile([P, D], FP, tag=bname)
                nc.gpsimd.dma_start(out=t, in_=_bcast_ap(dr[bname][:]))
                bc[bname] = t

        # ---------------- chunk registry + pump ----------------
        chunks = {}
        fillq = deque()
        deficit = [0.0]

        def reg(key, fn):
            chunks[key] = fn
            fillq.append(key)

        def need(key):
            fn = chunks.pop(key, None)
            if fn is not None:
                fn()

        def pump(ns):
            deficit[0] += ns
            while deficit[0] >= CHUNK_NS and fillq:
                key = fillq.popleft()
                fn = chunks.pop(key, None)
                if fn is None:
                    continue  # already forced via need()
                fn()
                deficit[0] -= CHUNK_NS

        def drainq():
            while fillq:
                key = fillq.popleft()
                fn = chunks.pop(key, None)
                if fn is not None:
                    fn()

        tiles = {}

        # ---------------- phase A: projections ----------------
        def dma_in(b):
            qt = io.tile([P, DT, N], BF, tag="qt", name="qt")
            kt = io.tile([P, DT, N], BF, tag="kt", name="kt")
            for c in range(DT):
                nc.sync.dma_start(out=kt[:, c, :], in_=kt_src[b, :, c, :])
            for c in range(DT):
                nc.sync.dma_start(out=qt[:, c, :], in_=qt_src[b, :, c, :])
            st = tiles.setdefault(b, {})
            st["qt"], st["kt"] = qt, kt

        def make_batch_tiles(b):
            st = tiles[b]
            st["qpt"] = big.tile([P, DT, N], BF, tag="qpt", name="qpt")
            st["kpt"] = big.tile([P, DT, N], BF, tag="kpt", name="kpt")
            st["vpa"] = big.tile([P, NT, H, HA], BF, tag="vpa", name="vpa")
            st["oasm"] = big.tile([P, NT, D], FP, tag="oasm", name="oasm")
            st["ln1"] = big.tile([P, NT, D], BF, tag="ln1", name="ln1")
            # denominator ones columns (evictions never touch col 64)
            nc.gpsimd.memset(st["vpa"][:, :, :, HD:HA], 1.0)

        def proj_chunk(b, wname, dst_name, bias_sb, t, hf):
            """One [128,512] tile of QpT/KpT: 4 matmuls + eviction."""

            def run():
                st = tiles[b]
                src = st["qt"] if wname == "Wq" else st["kt"]
                ps = ps_acc.tile([P, D], FP, tag="acc", name="projps")
                for c in range(DT):
                    nc.tensor.matmul(
                        ps,
                        wsb[wname][:, c, ts(t, P)],
                        src[:, c, ds(hf * 512, 512)],
                        start=(c == 0),
                        stop=(c == DT - 1),
                    )
                dst = st[dst_name][:, t, ds(hf * 512, 512)]
                if bias_sb is None:
                    nc.vector.tensor_copy(dst, ps)
                else:
                    nc.vector.tensor_scalar_add(dst, ps, bias_sb[:, t : t + 1])

            return run

        def vpa_chunk(b, m):
            def run():
                st = tiles[b]
                ps = ps_acc.tile([P, D], FP, tag="acc", name="vps")
                for c in range(DT):
                    nc.tensor.matmul(
                        ps,
                        st["kt"][:, c, ts(m, P)],
                        wsb["Wv"][:, c, :],
                        start=(c == 0),
                        stop=(c == DT - 1),
                    )
                vdst = st["vpa"][:, m, :, 0:HD]
                vsrc = ps[:, :].rearrange("p (h s) -> p h s", s=HD)
                if tbv:
                    nc.vector.tensor_copy(vdst, vsrc)
                else:
                    nc.vector.scalar_tensor_tensor(
                        out=vdst,
                        in0=vsrc,
                        scalar=0.0,
                        in1=bc["bv"][:, :].rearrange("p (h s) -> p h s", s=HD),
                        op0=ALU.bypass,
                        op1=ALU.add,
                    )

            return run

        def qp_chunk(b, m):
            """Prefill oasm[:, m, :] with natural-layout Qp (residual)."""

            def run():
                st = tiles[b]
                tp = ps_acc.tile([P, D], BF, tag="acc", name="qptr")
                for t in range(DT):
                    nc.tensor.transpose(
                        tp[:, ts(t, P)], st["qpt"][:, t, ts(m, P)], ident_bf
                    )
                nc.vector.tensor_copy(st["oasm"][:, m, :], tp)

            return run

        def phase_a_chunks(b):
            """(crit, rest) keyed chunk lists in need-order for batch b."""
            pq = lambda t, hf: (("qpt", b, t, hf), proj_chunk(b, "Wq", "qpt", bq_sb, t, hf))
            pk = lambda t, hf: (("kpt", b, t, hf), proj_chunk(b, "Wk", "kpt", bk_sb, t, hf))
            va = lambda m: (("va", b, m), vpa_chunk(b, m))
            qp = lambda m: (("qp", b, m), qp_chunk(b, m))
            crit = [pk(0, 0), pk(0, 1), pq(0, 0), va(0), va(1), va(2)]
            rest = [
                va(3),
                pq(1, 0),
                va(4),
                pq(2, 0),
                va(5),
                pq(3, 0),
                va(6),
                pk(1, 0),
                pk(1, 1),
                va(7),
                pk(2, 0),
                pk(2, 1),
                pk(3, 0),
                pk(3, 1),
                pq(0, 1),
                pq(1, 1),
                pq(2, 1),
                pq(3, 1),
                qp(0),
                qp(1),
                qp(2),
                qp(3),
                qp(4),
                qp(5),
                qp(6),
                qp(7),
            ]
            return crit, rest

        def force_qtkt_readers(b):
            """Emit all still-pending chunks that read qt/kt of batch b."""
            for key in [("kpt", b, t, hf) for t in range(DT) for hf in range(2)] + [
                ("qpt", b, t, hf) for t in range(DT) for hf in range(2)
            ] + [("va", b, m) for m in range(NT)]:
                need(key)

        # ---------------- LN1 + FFN ----------------
        def ln1_half(b, hf):
            st = tiles[b]
            mv = sml.tile([P, QH, 2], FP, tag="mva", name="mva")
            for i in range(QH):
                q = hf * QH + i
                bn = sml.tile([P, 6], FP, tag="bn", name="bn1")
                nc.vector.bn_stats(bn, st["oasm"][:, q, :])
                nc.vector.bn_aggr(mv[:, i, :], bn)
            rsa = sml.tile([P, QH], FP, tag="rsa", name="rsa")
            nc.scalar.activation(rsa, mv[:, :, 1], AF.Sqrt, bias=eps_sb)
            nc.vector.reciprocal(rsa, rsa)
            for i in range(QH):
                q = hf * QH + i
                lq = st["ln1"][:, q, :]
                nc.vector.tensor_scalar(
                    out=lq,
                    in0=st["oasm"][:, q, :],
                    scalar1=mv[:, i, 0:1],
                    scalar2=rsa[:, i : i + 1],
                    op0=ALU.subtract,
                    op1=ALU.mult,
                )
                if not triv0:
                    nc.vector.tensor_tensor(lq, lq, bc["g0"], ALU.mult)
                    nc.vector.tensor_tensor(lq, lq, bc["b0"], ALU.add)

        def ffn_chunk(b, q):
            def run():
                st = tiles[b]
                tp = ps_acc.tile([P, D], BF, tag="acc", name="lntr")
                for c in range(DT):
                    nc.tensor.transpose(
                        tp[:, ts(c, P)], st["ln1"][:, q, ts(c, P)], ident_bf
                    )
                l_t = two.tile([P, DT, P], BF, tag="lt", name="lt")
                nc.vector.tensor_copy(l_t, tp)

                f_ps = ps_acc.tile([P, D], FP, tag="acc", name="ffps")
                for c in range(DT):
                    nc.tensor.matmul(
                        f_ps,
                        l_t[:, c, :],
                        wsb["Wo"][:, c, :],
                        start=(c == 0),
                        stop=(c == DT - 1),
                    )
                o2 = two.tile([P, D], FP, tag="o2", name="o2")
                if trivbo:
                    # o2 = relu(ffn) + ln1 in one STT
                    nc.vector.scalar_tensor_tensor(
                        out=o2,
                        in0=f_ps,
                        scalar=0.0,
                        in1=st["ln1"][:, q, :],
                        op0=ALU.max,
                        op1=ALU.add,
                    )
                else:
                    rl = two.tile([P, D], FP, tag="rl", name="rl")
                    nc.vector.tensor_tensor(rl, f_ps, bc["bo"], ALU.add)
                    nc.scalar.activation(rl, rl, AF.Relu)
                    nc.vector.tensor_tensor(o2, rl, st["ln1"][:, q, :], ALU.add)

                bn = sml.tile([P, 6], FP, tag="bn", name="bn2")
                nc.vector.bn_stats(bn, o2)
                mv2 = sml.tile([P, 2], FP, tag="mv2", name="mv2")
                nc.vector.bn_aggr(mv2, bn)
                rs2 = sml.tile([P, 1], FP, tag="rs2", name="rs2")
                nc.scalar.activation(rs2, mv2[:, 1:2], AF.Sqrt, bias=eps_sb)
                nc.vector.reciprocal(rs2, rs2)
                z2 = two.tile([P, D], FP, tag="z2", name="z2")
                nc.vector.tensor_scalar(
                    out=z2,
                    in0=o2,
                    scalar1=mv2[:, 0:1],
                    scalar2=rs2,
                    op0=ALU.subtract,
                    op1=ALU.mult,
                )
                if not triv1:
                    nc.vector.tensor_tensor(z2, z2, bc["g1"], ALU.mult)
                    nc.vector.tensor_tensor(z2, z2, bc["b1"], ALU.add)
                nc.sync.dma_start(out=out_O[b, ts(q, P), :], in_=z2)

            return run

        # ---------------- phase B: attention ----------------
        def phase_b(b, at_hf0_end=None):
            st = tiles[b]
            qpt, kpt, vpa, oasm = st["qpt"], st["kpt"], st["vpa"], st["oasm"]
            pending = deque()  # (emit, p_tile, m)
            pending_drain = [None]

            def pop_pv():
                emit, p, m = pending.popleft()
                need(("va", b, m))
                emit(p, m)

            def mk_drain(hf, hp, pv0, pv1):
                def d():
                    for i in range(QH):
                        need(("qp", b, hf * QH + i))
                    for j, pv in ((0, pv0), (1, pv1)):
                        h = 2 * hp + j
                        r4 = sml.tile([P, QH], FP, tag="r4", name="r4")
                        nc.vector.reciprocal(r4, pv[:, :, HD:HA])
                        for qt in range(QH):
                            o_slice = oasm[:, hf * QH + qt, ds(h * HD, HD)]
                            nc.vector.scalar_tensor_tensor(
                                out=o_slice,
                                in0=pv[:, qt, 0:HD],
                                scalar=r4[:, qt : qt + 1],
                                in1=o_slice,
                                op0=ALU.mult,
                                op1=ALU.add,
                            )

                return d

            for hf in range(2):
                for hp in range(PAIRS):
                    need(("kpt", b, hp, 0))
                    need(("kpt", b, hp, 1))
                    need(("qpt", b, hp, hf))
                    pv0 = ps_pv.tile([P, QH, HA], FP, tag="pv", name="pv0")
                    pv1 = ps_pv.tile([P, QH, HA], FP, tag="pv", name="pv1")

                    def mk_emit(pv0=pv0, pv1=pv1, hp=hp):
                        def emit(p, m):
                            for j, pv in ((0, pv0), (1, pv1)):
                                for qt in range(QH):
                                    nc.tensor.matmul(
                                        pv[:, qt, :],
                                        p[:, ds(j * 512 + qt * P, P)],
                                        vpa[:, m, 2 * hp + j, :],
                                        start=(m == 0),
                                        stop=(m == NT - 1),
                                    )

                        return emit

                    emit = mk_emit()
                    for m in range(NT):
                        s = ps_flow.tile([P, N], FP, tag="flow", name="spair")
                        for j in range(2):
                            lo = j * HD
                            nc.tensor.matmul(
                                s[:, ds(j * 512, 512)],
                                kpt[lo : lo + HD, hp, ts(m, P)],
                                qpt[lo : lo + HD, hp, ds(hf * 512, 512)],
                                start=True,
                                stop=True,
                            )
                        p = pch.tile([P, N], BF, tag="p", name="p")
                        nc.scalar.activation(p, s, AF.Exp, scale=SCALE)
                        pending.append((emit, p, m))
                        if len(pending) > 2:
                            pop_pv()
                        if m == 1 and pending_drain[0] is not None:
                            pending_drain[0]()
                            pending_drain[0] = None
                        pump(STEP_IDLE_NS)
                    pending_drain[0] = mk_drain(hf, hp, pv0, pv1)

                # ---- end of this q-half: flush pipeline, drain, LN1, FFN
                while pending:
                    pop_pv()
                pending_drain[0]()
                pending_drain[0] = None
                ln1_half(b, hf)
                for i in range(QH):
                    q = hf * QH + i
                    reg(("ffn", b, q), ffn_chunk(b, q))
                if hf == 0 and at_hf0_end is not None:
                    at_hf0_end()

        # ---------------- main flow ----------------
        dma_in(0)
        nc.sync.dma_start(
            out=wsb["Wo"], in_=dr["Wo"][:].rearrange("(c p) d -> p c d", p=P)
        )
        make_batch_tiles(0)
        crit0, rest0 = phase_a_chunks(0)
        for _key, fn in crit0:
            fn()
        for key, fn in rest0:
            reg(key, fn)

        def start_b1():
            force_qtkt_readers(0)
            dma_in(1)
            make_batch_tiles(1)
            crit1, rest1 = phase_a_chunks(1)
            for key, fn in crit1 + rest1:
                reg(key, fn)

        phase_b(0, at_hf0_end=start_b1)
        phase_b(1)
        drainq()

    nc.compile()
    return nc


_NC = {}


def _get_nc(key):
    if key not in _NC:
        _NC[key] = _build_program(*key)
    return _NC[key]


def _prep_in_maps(inputs):
    import ml_dtypes

    f32 = lambda x: np.ascontiguousarray(np.asarray(x), dtype=np.float32)

    def bf(x):
        return np.ascontiguousarray(
            np.asarray(x, dtype=np.float32).astype(ml_dtypes.bfloat16)
        )

    Q, K = f32(inputs["Q"]), f32(inputs["K"])
    QT = np.ascontiguousarray(Q.transpose(0, 2, 1))
    KT = np.ascontiguousarray(K.transpose(0, 2, 1))
    shared = {
        "Wq": bf(inputs["Wq"]),
        "Wk": bf(inputs["Wk"]),
        "Wv": bf(inputs["Wv"]),
        "Wo": bf(inputs["Wo"]),
        "bq2": np.ascontiguousarray(f32(inputs["bq"]).reshape(DT, P).T),
        "bk2": np.ascontiguousarray(f32(inputs["bk"]).reshape(DT, P).T),
        "bv": f32(inputs["bv"]),
        "bo": f32(inputs["bo"]),
        "g0": f32(inputs["g0"]),
        "b0": f32(inputs["b0"]),
        "g1": f32(inputs["g1"]),
        "b1": f32(inputs["b1"]),
    }
    in_maps = []
    for c in range(NCORES):
        m = dict(shared)
        m["QT"] = np.ascontiguousarray(
            QT[c * BL : (c + 1) * BL].astype(ml_dtypes.bfloat16)
        )
        m["KT"] = np.ascontiguousarray(
            KT[c * BL : (c + 1) * BL].astype(ml_dtypes.bfloat16)
        )
        in_maps.append(m)
    return in_maps


def _run(inputs, trace=False):
    a = np.asarray
    key = (
        bool(np.all(a(inputs["bq"]) == 0.0)),
        bool(np.all(a(inputs["bk"]) == 0.0)),
        bool(np.all(a(inputs["bv"]) == 0.0)),
        bool(np.all(a(inputs["bo"]) == 0.0)),
        bool(np.all(a(inputs["g0"]) == 1.0) and np.all(a(inputs["b0"]) == 0.0)),
        bool(np.all(a(inputs["g1"]) == 1.0) and np.all(a(inputs["b1"]) == 0.0)),
    )
    nc = _get_nc(key)
    in_maps = _prep_in_maps(inputs)
    return run_bass_kernel_spmd(nc, in_maps, list(range(NCORES)), trace=trace)


def kernel(**inputs):
    res = _run(inputs, trace=False)
    return np.concatenate([res.results[c]["O"] for c in range(NCORES)], axis=0)
